# revision 30
# baseline (speedup 1.0000x reference)
"""Trainium2 Bass kernel for nn_LowBitMixIn.

Computes out[b,o,t] = sum_i mixer[o,i] * x[b, perm[i], t] for
x:[16,1024,4096] f32, mixer:[1024,1024] f32 (banded: 7 cyclic
sub-diagonals), perm:[1024] int32.

Strategy: data-parallel over batch (2 batches per core, 8 cores).
Host folds the permutation into indirect-DMA row-gather indices; the
band (signed diagonal span S) lets each output tile of M = 129-S rows
be computed from a single 128-row gathered input window with ONE K=128
matmul per 512-column PSUM chunk. Everything on-device runs bf16
(tolerance is 2e-2; mixer entries are powers of two, exact in bf16),
halving DMA bytes and packet count vs fp32.

If the mixer is not banded (verified by exact reconstruction), a
generic fp32 block-dense path is used instead.
"""

import os
import sys

import numpy as np
import ml_dtypes

sys.path.insert(0, "/opt/trn_rl_repo")

from concourse import bacc, bass, mybir, tile  # noqa: E402
from concourse.bass_utils import run_bass_kernel_spmd  # noqa: E402

BF16 = np.dtype(ml_dtypes.bfloat16)

F = 1024
T = 4096
B = 16
N_CORES = 8
B_SHARD = B // N_CORES
NCHUNK = 512  # PSUM bank: 512 fp32 entries per partition

_PROGRAM_CACHE = {}
LAST_RESULTS = None  # test harness introspection
LAST_NC = None
LAST_IN_MAPS = None


def _build_banded_program(b_shard, f, t, tile_m, n_tiles, reps=1, nsp=6,
                          wq="sw", xt_bufs=3, ot_bufs=5, ps_bufs=8,
                          gsplit=1, csplit=1, pack_w=True, pdepth=0,
                          wstride=False, tailq="sw", dt=mybir.dt.bfloat16):
    """wq: which DGE queue carries the output writes.
    'sw'   = gpsimd SWDGE (ucode descriptor gen, 16-engine spread)
    'sync' = sync-engine HWDGE queue (hardware descriptor expansion)
    'ss'   = alternate the two HWDGE rings
    nsp: row-splits per output tile write; gsplit: T-splits per gather;
    csplit: 2 = both copy engines take half the rows of every chunk;
    pack_w: load all stationary weights with ONE wide DMA (the per-tile
    loads cost ~10us of ramp in 244B descriptors);
    pdepth: software-pipeline the EMISSION so each tile's gather is
    issued pdepth tiles before its compute/write — otherwise the
    write's sem-wait on the in-order gpsimd stream blocks the next
    gather and caps effective prefetch at ~1 tile."""
    nc = bacc.Bacc()
    x_in = nc.declare_dram_parameter("x", [b_shard * f, t], dt, isOutput=False)
    if pack_w:
        wts_in = nc.declare_dram_parameter(
            "wts", [128, n_tiles * tile_m], dt, isOutput=False
        )
    else:
        wts_in = nc.declare_dram_parameter(
            "wts", [n_tiles, 128, tile_m], dt, isOutput=False
        )
    gidx_in = nc.declare_dram_parameter(
        "gidx", [128, b_shard * n_tiles], mybir.dt.int32, isOutput=False
    )
    out_ext = nc.declare_dram_parameter(
        "out", [b_shard * f, t], dt, isOutput=True
    )
    n_chunks = t // NCHUNK
    with tile.TileContext(nc) as tc:
        with (
            tc.tile_pool(name="const", bufs=1) as cpool,
            tc.tile_pool(name="xp", bufs=xt_bufs) as xpool,
            tc.tile_pool(name="op", bufs=ot_bufs) as opool,
            tc.tile_pool(name="ps", bufs=ps_bufs, space="PSUM") as pspool,
        ):
            idx_tile = cpool.tile([128, b_shard * n_tiles], mybir.dt.int32)
            # SWDGE ring so the first gather isn't gated on sync-ring
            # init + table loads (~6us of ramp)
            nc.gpsimd.dma_start(out=idx_tile[:], in_=gidx_in[:])
            if pack_w:
                wpacked = cpool.tile([128, n_tiles * tile_m], dt, tag="wp")
                nc.sync.dma_start(out=wpacked[:], in_=wts_in[:])
                w_tiles = [
                    wpacked[:, p * tile_m : (p + 1) * tile_m]
                    for p in range(n_tiles)
                ]
            else:
                w_tiles = []
                for p in range(n_tiles):
                    wt = cpool.tile([128, tile_m], dt, tag=f"w{p}")
                    nc.sync.dma_start(out=wt[:], in_=wts_in[p])
                    w_tiles.append(wt[:])
            for _rep in range(reps):
                # partial tiles (smallest write volume) go LAST so the
                # final DRAIN waits on as little outstanding data as
                # possible
                work = sorted(
                    ((bi, p) for bi in range(b_shard)
                     for p in range(n_tiles)),
                    key=lambda w: f - w[1] * tile_m < tile_m,
                )
                staged = []
                n_work = len(work)
                done = 0
                for bi, p in work:
                    xt = _emit_gather(
                        nc, t, bi, p, n_tiles, idx_tile, xpool, x_in,
                        gsplit, dt,
                    )
                    staged.append((bi, p, xt))
                    if len(staged) > pdepth:
                        _emit_compute_write(
                            nc, f, t, tile_m, n_tiles, *staged.pop(0),
                            w_tiles, opool, pspool, out_ext,
                            nsp, wq, csplit, dt, done >= n_work - 2,
                            wstride, tailq,
                        )
                        done += 1
                for item in staged:
                    _emit_compute_write(
                        nc, f, t, tile_m, n_tiles, *item,
                        w_tiles, opool, pspool, out_ext,
                        nsp, wq, csplit, dt, done >= n_work - 2,
                        wstride, tailq,
                    )
                    done += 1
    return nc


def _emit_gather(nc, t, bi, p, n_tiles, idx_tile, xpool, x_in, gsplit, dt):
    col = bi * n_tiles + p
    xt = xpool.tile([128, t], dt, tag="xt")
    tg = t // gsplit
    for gi in range(gsplit):
        nc.gpsimd.indirect_dma_start(
            out=xt[:, gi * tg : (gi + 1) * tg],
            out_offset=None,
            in_=x_in[:],
            in_offset=bass.IndirectOffsetOnAxis(
                ap=idx_tile[:, col : col + 1], axis=0
            ),
            element_offset=gi * tg,
            bounds_check=x_in.shape[0] - 1,
            oob_is_err=False,
        )
    return xt


def _emit_compute_write(
    nc, f, t, tile_m, n_tiles, bi, p, xt,
    w_tiles, opool, pspool, out_ext, nsp, wq, csplit, dt, is_last=False,
    wstride=False, tailq="sw",
):
    n_chunks = t // NCHUNK
    o0 = p * tile_m
    m_p = min(tile_m, f - o0)
    k_need = m_p + (129 - tile_m) - 1  # window rows with nonzero weights
    ot = opool.tile([128, t], dt, tag="ot")
    for ni in range(n_chunks):
        sl = slice(ni * NCHUNK, (ni + 1) * NCHUNK)
        ps = pspool.tile([128, NCHUNK], mybir.dt.float32)
        nc.tensor.matmul(
            out=ps[:m_p, :],
            lhsT=w_tiles[p][:k_need, :m_p],
            rhs=xt[:k_need, sl],
            start=True,
            stop=True,
        )
        if csplit == 2:
            # both PSUM-drain engines take half the rows of every chunk
            # (Activation AP base partition must be 0/32/64)
            mh = 64
            nc.vector.tensor_copy(out=ot[:mh, sl], in_=ps[:mh, :])
            nc.scalar.copy(out=ot[mh:m_p, sl], in_=ps[mh:m_p, :])
        else:
            # alternate PSUM->SBUF drain across DVE and Activation
            eng = nc.vector.tensor_copy if ni % 2 == 0 else nc.scalar.copy
            eng(out=ot[:m_p, sl], in_=ps[:m_p, :])
        if is_last and ni % 2 == 1:
            # final tiles: column-split writes issue as soon as their two
            # chunks are drained, routed via the sync HWDGE ring so the
            # gpsimd ring's tail DRAIN has nothing left outstanding (the
            # two rings drain concurrently; HWDGE FIFO serialization is
            # fine for this small volume)
            c0, c1 = (ni - 1) * NCHUNK, (ni + 1) * NCHUNK
            tweng = nc.sync if tailq == "sync" else nc.gpsimd
            tweng.dma_start(
                out=out_ext[bi * f + o0 : bi * f + o0 + m_p, c0:c1],
                in_=ot[:m_p, c0:c1],
            )
    if is_last:
        return
    if wstride:
        # row-strided splits: every write's partitions span 0..127, so
        # each engages all 16 SDMA engines (the engine<->partition
        # swizzle maps contiguous 24-row ranges onto only ~7 engines)
        r0 = bi * f + o0
        for ri in range(nsp):
            nc.gpsimd.dma_start(
                out=out_ext[r0 + ri : r0 + m_p : nsp, :],
                in_=ot[ri : m_p : nsp, :],
            )
        return
    rb = [m_p * ri // nsp for ri in range(nsp + 1)]
    for ri in range(nsp):
        if wq == "sw":
            weng = nc.gpsimd
        elif wq == "sync":
            weng = nc.sync
        else:  # 'ss': alternate the two HWDGE rings (SP / Activation)
            weng = nc.sync if ri % 2 == 0 else nc.scalar
        weng.dma_start(
            out=out_ext[bi * f + o0 + rb[ri] : bi * f + o0 + rb[ri + 1], :],
            in_=ot[rb[ri] : rb[ri + 1], :],
        )


def _build_dense_program(b_shard, f, t):
    """Fallback: generic block matmul out_p = sum_q M[p,q] @ xp_q.

    Splits T in halves to fit 8 resident gathered input tiles in SBUF.
    """
    nc = bacc.Bacc()
    nq = f // 128
    x_in = nc.declare_dram_parameter("x", [b_shard * f, t], mybir.dt.float32, isOutput=False)
    wts_in = nc.declare_dram_parameter(
        "wts", [nq, nq, 128, 128], mybir.dt.float32, isOutput=False
    )
    gidx_in = nc.declare_dram_parameter(
        "gidx", [128, b_shard * nq], mybir.dt.int32, isOutput=False
    )
    out_ext = nc.declare_dram_parameter(
        "out", [b_shard * f, t], mybir.dt.float32, isOutput=True
    )
    t_half = t // 2
    n_chunks = t_half // NCHUNK
    with tile.TileContext(nc) as tc:
        with (
            tc.tile_pool(name="const", bufs=1) as cpool,
            tc.tile_pool(name="xp", bufs=10) as xpool,
            tc.tile_pool(name="op", bufs=2) as opool,
            tc.tile_pool(name="ps", bufs=6, space="PSUM") as pspool,
        ):
            idx_tile = cpool.tile([128, b_shard * nq], mybir.dt.int32)
            nc.sync.dma_start(out=idx_tile[:], in_=gidx_in[:])
            w_tiles = {}
            for p in range(nq):
                for q in range(nq):
                    wt = cpool.tile([128, 128], mybir.dt.float32, tag=f"w{p}_{q}")
                    nc.sync.dma_start(out=wt[:], in_=wts_in[p, q])
                    w_tiles[(p, q)] = wt
            for bi in range(b_shard):
                for th in range(2):
                    t0 = th * t_half
                    xts = []
                    for q in range(nq):
                        col = bi * nq + q
                        xt = xpool.tile([128, t_half], mybir.dt.float32, tag="xt")
                        nc.gpsimd.indirect_dma_start(
                            out=xt[:],
                            out_offset=None,
                            in_=x_in[:],
                            in_offset=bass.IndirectOffsetOnAxis(
                                ap=idx_tile[:, col : col + 1], axis=0
                            ),
                            element_offset=t0,
                        )
                        xts.append(xt)
                    for p in range(nq):
                        ot = opool.tile([128, t_half], mybir.dt.float32, tag="ot")
                        for ni in range(n_chunks):
                            ps = pspool.tile([128, NCHUNK], mybir.dt.float32)
                            for q in range(nq):
                                nc.tensor.matmul(
                                    out=ps[:, :],
                                    lhsT=w_tiles[(p, q)][:],
                                    rhs=xts[q][:, ni * NCHUNK : (ni + 1) * NCHUNK],
                                    start=(q == 0),
                                    stop=(q == nq - 1),
                                )
                            nc.vector.tensor_copy(
                                out=ot[:, ni * NCHUNK : (ni + 1) * NCHUNK],
                                in_=ps[:, :],
                            )
                        nc.sync.dma_start(
                            out=out_ext[
                                bi * f + p * 128 : bi * f + (p + 1) * 128,
                                t0 : t0 + t_half,
                            ],
                            in_=ot[:, :],
                        )
    return nc


def _analyze(mixer, permutation, b_shard, f):
    """Derive band structure + weights/indices. Returns (mode, tile_m,
    n_tiles, wts, gidx)."""
    perm = permutation.astype(np.int64)
    o_idx, c_idx = np.nonzero(mixer)
    if len(o_idx) == 0:
        d_lo = d_hi = 0
    else:
        d = (o_idx - c_idx) % f
        d_signed = np.where(d > f // 2, d - f, d)
        d_lo, d_hi = int(d_signed.min()), int(d_signed.max())
    span = d_hi - d_lo + 1
    if span <= 128:
        tile_m = 129 - span
        n_tiles = -(-f // tile_m)
        wts = np.zeros((n_tiles, 128, tile_m), np.float32)
        gidx = np.zeros((128, b_shard * n_tiles), np.int32)
        a_hat = np.zeros((f, f), np.float32)
        k_arange = np.arange(128)
        for p in range(n_tiles):
            o0 = p * tile_m
            m_p = min(tile_m, f - o0)
            rows = (o0 - d_hi + k_arange) % f  # feature index i per window row
            wts[p, :, :m_p] = mixer[np.ix_(range(o0, o0 + m_p), rows)].T
            a_hat[np.ix_(range(o0, o0 + m_p), rows)] = wts[p, :, :m_p].T
            k_need = m_p + (129 - tile_m) - 1  # window rows actually used
            for bi in range(b_shard):
                col = bi * n_tiles + p
                gidx[:, col] = bi * f + perm[rows]
                # out-of-bounds sentinel: gather skips these rows entirely
                gidx[k_need:, col] = b_shard * f
        if np.array_equal(a_hat, mixer):
            return ("banded", tile_m, n_tiles, wts, gidx)
    # dense fallback
    nq = f // 128
    wts = np.ascontiguousarray(
        mixer.reshape(nq, 128, nq, 128).transpose(0, 2, 3, 1), dtype=np.float32
    )
    gidx = np.zeros((128, b_shard * nq), np.int32)
    for bi in range(b_shard):
        for q in range(nq):
            gidx[:, bi * nq + q] = bi * f + perm[q * 128 : (q + 1) * 128]
    return ("dense", 128, nq, wts, gidx)


# best measured config (sweeps 1-5: nsp=5 write splits on the SWDGE ring,
# gathers emitted 4 tiles ahead of their compute/write, weights packed
# into one DMA; 141us vs 298us fp32 staged baseline)
BEST = dict(nsp=5, wq="sw", xt_bufs=6, ot_bufs=9, ps_bufs=8, gsplit=1,
            csplit=1, pack_w=True, pdepth=4, wstride=True, tailq="sync")


def _pack_wts(wts, n_tiles, tile_m):
    """[n_tiles,128,tile_m] -> [128, n_tiles*tile_m] column-concat."""
    return np.ascontiguousarray(
        wts.transpose(1, 0, 2).reshape(128, n_tiles * tile_m)
    )


def kernel(x, mixer, permutation):
    global LAST_RESULTS
    x = np.ascontiguousarray(x, dtype=np.float32)
    mixer = np.asarray(mixer, dtype=np.float32)
    permutation = np.asarray(permutation)
    b, f, t = x.shape
    b_shard = b // N_CORES

    mode, tile_m, n_tiles, wts, gidx = _analyze(mixer, permutation, b_shard, f)

    bf16 = mode == "banded"
    key = (mode, b_shard, f, t, tile_m, n_tiles)
    if key not in _PROGRAM_CACHE:
        if mode == "banded":
            _PROGRAM_CACHE[key] = _build_banded_program(
                b_shard, f, t, tile_m, n_tiles, **BEST
            )
        else:
            _PROGRAM_CACHE[key] = _build_dense_program(b_shard, f, t)
    nc = _PROGRAM_CACHE[key]
    if not getattr(nc, "_lowbit_compiled", False):
        nc.compile()
        nc._lowbit_compiled = True

    x_dev = x.astype(BF16) if bf16 else x
    if bf16:
        wts_dev = wts.astype(BF16)
        if BEST.get("pack_w", False):
            wts_dev = _pack_wts(wts_dev, n_tiles, tile_m)
    else:
        wts_dev = wts
    in_maps = []
    for i in range(N_CORES):
        m = {
            "x": np.ascontiguousarray(
                x_dev[i * b_shard : (i + 1) * b_shard].reshape(b_shard * f, t)
            ),
            "wts": wts_dev,
            "gidx": gidx,
        }
        in_maps.append(m)
    global LAST_NC, LAST_IN_MAPS
    LAST_NC = nc
    LAST_IN_MAPS = in_maps
    res = run_bass_kernel_spmd(nc, in_maps, list(range(N_CORES)))
    LAST_RESULTS = res
    out = np.concatenate(
        [np.asarray(r["out"]).astype(np.float32).reshape(b_shard, f, t)
         for r in res.results],
        axis=0,
    )
    return out
